# revision 13
# baseline (speedup 1.0000x reference)
"""Trainium2 Bass kernel for MultiHeadCrossAttention (B=8,N=8,Q=128,K=1024,D=512,H=8).

Sharding: data-parallel over batch B — core i handles batch i.

Optimizations vs the naive form:
  1. Key compaction: `key_mask` is known at staging time and zeroes ~50% of
     keys, so the host compacts K=1024 -> KC=576 (max valid count is 550 for
     these inputs; KC re-picked at runtime if ever larger).  K-proj, V-proj,
     QK and AV all shrink by KC/K with no precision loss.
  2. The Gaussian distance bias is injected into the logits PSUM by a rank-12
     bf16 matmul: -d^2/(2s^2) = pq.pk/s^2 - |pk|^2/(2s^2) - |pq|^2/(2s^2);
     the per-query term is dropped (softmax is shift-invariant) and the
     remaining rank-4 bilinear form is expanded hi/lo (3 cross products of
     bf16 splits) for f32-faithful accuracy.  Padded key slots get a -1000
     bias column -> exp gives exactly 0.  This removes the big [Q,KC]
     exp-bias multiply on VectorE and lets the exp run straight out of PSUM
     with a fused row-sum (accum_out), so softmax sums are free.
  3. V bias is folded into the output-projection bias on the host
     (attn rows sum to 1), so the V-proj PSUM evacuation is a plain copy.
  4. Attention rows are transposed for AV with one DMA-xbar transpose per
     head (padded to 640 cols so key chunks stay 128-aligned), alternating
     between the two HWDGE queues (sync/scalar) so the transposes ride
     otherwise-idle DMA paths.

Per-core dataflow (matmuls bf16 on TensorE, f32 PSUM accumulate):
  - host stages compacted transposed bf16 activations (kvT [D, N*KC],
    qT [D, N*Q]) and transposed bf16 weights; SCALE folded into Wq/bq;
    rank-12 bias factors aq12 [12, Q] / ak12 [12, KC] duplicated at
    partition 32 for row-group-concurrent matmuls.
  - Q-proj once up front -> qTp [j, m] (transposed layout, heads on partitions)
  - per step n: K-proj -> kT [j, k] (transposed), V-proj -> v [k, j] (natural)
  - per head pair: logits = qT'^T @ kT + rank-12 bias (PSUM, natural [q, k]);
    exp on ScalarE with fused row-sum; normalize by 1/rowsum on VectorE;
    DMA-xbar transpose attn -> [k, q]; AV accumulated over key chunks
    (col-group interleaved pairs), delayed one step to hide the transpose.
  - out-proj: 4 accumulating matmuls, bias added during final evacuation.
"""

import numpy as np
import ml_dtypes

B, N, Q, K, D, H = 8, 8, 128, 1024, 512, 8
HD = D // H
SCALE = HD ** -0.5
SIGMA2 = max(0.35 * 0.35, 1e-6)
NCORES = 8
KC = 640           # compacted key count (multiple of 128; >= max valid keys)

_BF16 = ml_dtypes.bfloat16

_CACHE = {}


def _build_program(repeat=1, kc=KC):
    import concourse.bass as bass
    import concourse.mybir as mybir
    import concourse.tile as tile
    from concourse import bacc

    f32 = mybir.dt.float32
    bf16 = mybir.dt.bfloat16
    AF = mybir.ActivationFunctionType
    ALU = mybir.AluOpType

    # key-axis chunking
    kb_chunks = [(0, 512), (512, kc - 512)] if kc > 512 else [(0, kc)]
    nvc = (kc + 127) // 128          # 128-key chunks (last may be partial)
    last_w = kc - 128 * (nvc - 1)    # valid width of last chunk
    ph = 128 * nvc                   # per-head padded attn width (transpose)

    nc = bacc.Bacc("TRN2", target_bir_lowering=False, debug=False,
                   num_devices=NCORES)

    kvT_h = nc.declare_dram_parameter("kvT", [D, N * kc], bf16, isOutput=False)
    qT_h = nc.declare_dram_parameter("qT", [D, N * Q], bf16, isOutput=False)
    wq_h = nc.declare_dram_parameter("wqT", [D, D], bf16, isOutput=False)
    wk_h = nc.declare_dram_parameter("wkT", [D, D], bf16, isOutput=False)
    wv_h = nc.declare_dram_parameter("wvT", [D, D], bf16, isOutput=False)
    wo_h = nc.declare_dram_parameter("woT", [D, D], bf16, isOutput=False)
    bq_h = nc.declare_dram_parameter("bq2", [128, 4], f32, isOutput=False)
    bk_h = nc.declare_dram_parameter("bk2", [128, 4], f32, isOutput=False)
    bo_h = nc.declare_dram_parameter("bob", [128, D], f32, isOutput=False)
    aq_h = nc.declare_dram_parameter("aq12", [64, Q], bf16, isOutput=False)
    ak_h = nc.declare_dram_parameter("ak12", [64, kc], bf16, isOutput=False)
    out_h = nc.declare_dram_parameter("out", [N, Q, D], f32, isOutput=True)

    kvT = kvT_h.ap().rearrange("(c p) m -> p c m", p=128)   # [128, 4, N*kc]
    qT = qT_h.ap().rearrange("(c p) m -> p c m", p=128)     # [128, 4, N*Q]
    w_aps = {k: h.ap().rearrange("(c p) j -> p c j", p=128)
             for k, h in (("wq", wq_h), ("wk", wk_h), ("wv", wv_h), ("wo", wo_h))}
    out_ap = out_h.ap()

    with tile.TileContext(nc) as tc:
        with (
            tc.tile_pool(name="const", bufs=1) as cpool,
            tc.tile_pool(name="kvin", bufs=3) as kvpool,
            tc.tile_pool(name="kt", bufs=2) as ktpool,
            tc.tile_pool(name="vt", bufs=3) as vtpool,
            tc.tile_pool(name="attn", bufs=4) as apool,
            tc.tile_pool(name="abT", bufs=9) as tpool,
            tc.tile_pool(name="small", bufs=18) as spool,
            tc.tile_pool(name="oav", bufs=2) as opool,
            tc.tile_pool(name="pp", bufs=2, space="PSUM") as pp,
            tc.tile_pool(name="pl", bufs=2, space="PSUM") as pl,
            tc.tile_pool(name="pav", bufs=2, space="PSUM") as pav,
        ):
            # ---- constants (critical-path loads first) ----
            w = {}
            for name in ("wq", "wk", "wv", "wo"):
                w[name] = cpool.tile([128, 4, D], bf16, tag=name, name=name)
            qin = cpool.tile([128, 4, N * Q], bf16, tag="qin", name="qin")
            nc.gpsimd.dma_start(out=w["wq"][:], in_=w_aps["wq"][:])
            for mb in range(2):
                nc.gpsimd.dma_start(out=qin[:, :, mb * 512:(mb + 1) * 512],
                                    in_=qT[:, :, mb * 512:(mb + 1) * 512])
            bq2 = cpool.tile([128, 4], f32, tag="bq2", name="bq2")
            nc.gpsimd.dma_start(out=bq2[:], in_=bq_h.ap()[:])
            nc.gpsimd.dma_start(out=w["wk"][:], in_=w_aps["wk"][:])
            bk2 = cpool.tile([128, 4], f32, tag="bk2", name="bk2")
            nc.gpsimd.dma_start(out=bk2[:], in_=bk_h.ap()[:])
            nc.gpsimd.dma_start(out=w["wv"][:], in_=w_aps["wv"][:])
            aq12 = cpool.tile([64, Q], bf16, tag="aq12", name="aq12")
            nc.gpsimd.dma_start(out=aq12[:], in_=aq_h.ap()[:])
            ak12 = cpool.tile([64, kc], bf16, tag="ak12", name="ak12")
            nc.gpsimd.dma_start(out=ak12[:], in_=ak_h.ap()[:])
            bob = cpool.tile([128, D], f32, tag="bob", name="bob")

            # ---- Q projection for all steps ----
            qTp = cpool.tile([128, 4, N * Q], bf16, tag="qTp", name="qTp")

            def qproj_unit(jc, mb):
                ps = pp.tile([128, 512], f32, tag="pp", name="pp")
                for ic in range(4):
                    nc.tensor.matmul(
                        ps[:],
                        w["wq"][:, ic, jc * 128:(jc + 1) * 128],
                        qin[:, ic, mb * 512:(mb + 1) * 512],
                        start=(ic == 0), stop=(ic == 3),
                    )
                nc.scalar.activation(
                    out=qTp[:, jc, mb * 512:(mb + 1) * 512], in_=ps[:],
                    func=AF.Identity, bias=bq2[:, jc:jc + 1])

            for jc in range(4):
                for mb in range(2):
                    qproj_unit(jc, mb)

            def load_kv(n, parts=2):
                t = kvpool.tile([128, 4, kc], bf16, tag="kvin", name=f"kvin{n}")
                step = kc // parts
                for hh in range(parts):
                    nc.gpsimd.dma_start(
                        out=t[:, :, hh * step:(hh + 1) * step],
                        in_=kvT[:, :, n * kc + hh * step:n * kc + (hh + 1) * step])
                return t

            def kproj_unit(kvin, kt, jc, mb):
                off, wdt = kb_chunks[mb]
                ps = pp.tile([128, 512], f32, tag="pp", name="pp")
                for ic in range(4):
                    nc.tensor.matmul(
                        ps[:, 0:wdt],
                        w["wk"][:, ic, jc * 128:(jc + 1) * 128],
                        kvin[:, ic, off:off + wdt],
                        start=(ic == 0), stop=(ic == 3),
                    )
                nc.scalar.activation(
                    out=kt[:, jc, off:off + wdt], in_=ps[:, 0:wdt],
                    func=AF.Identity, bias=bk2[:, jc:jc + 1])

            pending_evacs = []

            def flush_evacs():
                while pending_evacs:
                    ps, vt, mc, wdt = pending_evacs.pop(0)
                    nc.vector.tensor_copy(out=vt[0:wdt, mc, :],
                                          in_=ps[0:wdt, :])

            def vproj_unit(kvin, vt, mc):
                # defer the PSUM->SBUF cast past the next attention pair's
                # DVE ops (reciprocal/normalize) to avoid head-of-line
                # blocking in the VectorE FIFO; vt has a full step of slack.
                if pending_evacs:
                    flush_evacs()
                wdt = 128 if mc < nvc - 1 else last_w
                ps = pp.tile([128, 512], f32, tag="pp", name="pp")
                for ic in range(4):
                    nc.tensor.matmul(
                        ps[0:wdt, :],
                        kvin[:, ic, mc * 128:mc * 128 + wdt],
                        w["wv"][:, ic, :],
                        start=(ic == 0), stop=(ic == 3),
                    )
                pending_evacs.append((ps, vt, mc, wdt))

            def kproj(kvin):
                kt = ktpool.tile([128, 4, kc], bf16, tag="kt", name="kt")
                for jc in range(4):
                    for mb in range(len(kb_chunks)):
                        kproj_unit(kvin, kt, jc, mb)
                return kt

            def vproj(kvin):
                vt = vtpool.tile([128, nvc, D], bf16, tag="vt", name="vt")
                for mc in range(nvc):
                    vproj_unit(kvin, vt, mc)
                return vt

            def qk_softmax_pair(n, c, kt):
                """QK + rank-12 bias for head pair (2c, 2c+1) with
                row-group-interleaved matmuls, exp with fused row-sum,
                1/sum normalize; one xbar transpose per head (the two
                heads alternate between the sync and scalar HWDGE rings)."""
                psls = [pl.tile([Q, 1024], f32, tag="pl", name="pl")
                        for _ in range(2)]
                for off, wdt in kb_chunks:
                    for par in range(2):
                        e = par * 64
                        nc.tensor.matmul(
                            psls[par][:, off:off + wdt],
                            qTp[e:e + 64, c, n * Q:(n + 1) * Q],
                            kt[e:e + 64, c, off:off + wdt],
                            start=True, stop=False,
                            skip_group_check=True,
                        )
                for off, wdt in kb_chunks:
                    for par in range(2):
                        b0 = par * 32
                        nc.tensor.matmul(
                            psls[par][:, off:off + wdt],
                            aq12[b0:b0 + 12, :],
                            ak12[b0:b0 + 12, off:off + wdt],
                            start=False, stop=True,
                            skip_group_check=True,
                        )
                abTs = []
                for par in range(2):
                    ab = apool.tile([Q, kc], bf16, tag="ab", name="ab")
                    sums = spool.tile([Q, 1], f32, tag="sums", name="sums")
                    nc.scalar.activation(out=ab[:], in_=psls[par][:, 0:kc],
                                         func=AF.Exp, accum_out=sums[:])
                    rec = spool.tile([Q, 1], f32, tag="rec", name="rec")
                    nc.vector.reciprocal(rec[:], sums[:])
                    abn = apool.tile([Q, ph], bf16, tag="abn", name="abn")
                    nc.vector.tensor_scalar_mul(
                        out=abn[:, 0:kc], in0=ab[:], scalar1=rec[:])
                    abT = tpool.tile([128, nvc, Q], bf16, tag="abT",
                                     name="abT")
                    nc.sync.dma_start_transpose(abT[:], abn[:])
                    abTs.append(abT)
                return abTs

            def av_pair(hc, abT2, vt, oavT):
                """AV for head pair (2hc, 2hc+1), col-group interleaved."""
                psav = pav.tile([128, Q], f32, tag="pav", name="pav")
                for cidx in range(nvc):
                    wdt = 128 if cidx < nvc - 1 else last_w
                    for par in range(2):
                        h = 2 * hc + par
                        e = par * 64
                        nc.tensor.matmul(
                            psav[e:e + HD, :],
                            vt[0:wdt, cidx, h * HD:(h + 1) * HD],
                            abT2[par][0:wdt, cidx, :],
                            start=(cidx == 0), stop=(cidx == nvc - 1),
                            skip_group_check=True,
                        )
                nc.vector.tensor_copy(
                    out=oavT[:, hc, :], in_=psav[:])

            def outproj(n, oavT):
                pso = pp.tile([Q, D], f32, tag="pp", name="pp")
                for jc in range(4):
                    nc.tensor.matmul(
                        pso[:], oavT[:, jc, :], w["wo"][:, jc, :],
                        start=(jc == 0), stop=(jc == 3))
                osb = opool.tile([Q, D], f32, tag="osb", name="osb")
                nc.vector.scalar_tensor_tensor(
                    out=osb[:], in0=pso[:], scalar=1.0, in1=bob[:],
                    op0=ALU.mult, op1=ALU.add)
                nc.gpsimd.dma_start(out=out_ap[n], in_=osb[:])

            # ---- software-pipelined steps (AV delayed one step) ----
            for _rep in range(repeat):
                kv0 = load_kv(0, parts=4)
                kv1 = load_kv(1)
                kvs = {0: kv0, 1: kv1}
                kts = {0: kproj(kv0)}
                vts = {0: vproj(kv0)}
                flush_evacs()
                abTs_prev = None
                for n in range(N):
                    if n + 2 < N:
                        kvs[n + 2] = load_kv(n + 2)
                    if _rep == 0 and n == 0:
                        nc.gpsimd.dma_start(out=w["wo"][:], in_=w_aps["wo"][:])
                        nc.gpsimd.dma_start(out=bob[:], in_=bo_h.ap()[:])
                    if n + 1 < N:
                        ktn = ktpool.tile([128, 4, kc], bf16, tag="kt",
                                          name="kt")
                        vtn = vtpool.tile([128, nvc, D], bf16, tag="vt",
                                          name="vt")
                        kus = [(kproj_unit, (kvs[n + 1], ktn, jc, mb))
                               for jc in range(4)
                               for mb in range(len(kb_chunks))]
                        vus = [(vproj_unit, (kvs[n + 1], vtn, mc))
                               for mc in range(nvc)]
                        proj_units = []
                        while kus or vus:
                            proj_units.extend(kus[:2]); kus = kus[2:]
                            proj_units.extend(vus[:1]); vus = vus[1:]
                        kts[n + 1] = ktn
                        vts[n + 1] = vtn
                    else:
                        proj_units = []
                    oavT_prev = None
                    if abTs_prev is not None:
                        oavT_prev = opool.tile([128, 4, Q], bf16,
                                               tag="oavT", name="oavT")
                    abTs = []
                    pu = 0
                    for hc in range(4):
                        abTs.append(qk_softmax_pair(n, hc, kts[n]))
                        flush_evacs()
                        if oavT_prev is not None:
                            av_pair(hc, abTs_prev[hc], vts[n - 1], oavT_prev)
                        take = 3 if hc < 3 else len(proj_units) - pu
                        for _ in range(max(0, take)):
                            if pu < len(proj_units):
                                fn, args = proj_units[pu]
                                fn(*args)
                                pu += 1
                    flush_evacs()
                    if oavT_prev is not None:
                        outproj(n - 1, oavT_prev)
                    abTs_prev = abTs
                # epilogue: AV + out-proj for the last step
                oavT_last = opool.tile([128, 4, Q], bf16, tag="oavT",
                                       name="oavT")
                for hc in range(4):
                    av_pair(hc, abTs_prev[hc], vts[N - 1], oavT_last)
                outproj(N - 1, oavT_last)

    nc.compile()
    return nc


def _bias_factors(query_pos, key_pos_valid, nv, kc):
    """Rank-12 bf16 factorization of the Gaussian bias (per-query term
    dropped; softmax-invariant).  Returns aq12 [64, Q], ak12 [64, kc]."""
    pq = query_pos - 0.5                     # [Q, 3] centered
    pk = key_pos_valid - 0.5                 # [nv, 3]
    aq4 = np.zeros((4, Q), np.float32)
    aq4[0:3] = pq.T / SIGMA2
    aq4[3] = 1.0
    ak4 = np.zeros((4, kc), np.float32)
    ak4[0:3, :nv] = pk.T
    ak4[3, :nv] = -(pk ** 2).sum(-1) / (2.0 * SIGMA2)
    ak4[3, nv:] = -1000.0                    # padded keys -> exp() == 0
    aq_hi = aq4.astype(_BF16).astype(np.float32)
    aq_lo = (aq4 - aq_hi).astype(_BF16).astype(np.float32)
    ak_hi = ak4.astype(_BF16).astype(np.float32)
    ak_lo = (ak4 - ak_hi).astype(_BF16).astype(np.float32)
    # bias ~= hi.hi + hi.lo + lo.hi
    aq12 = np.concatenate([aq_hi, aq_hi, aq_lo], 0)     # [12, Q]
    ak12 = np.concatenate([ak_hi, ak_lo, ak_hi], 0)     # [12, kc]
    aqt = np.zeros((64, Q), np.float32)
    akt = np.zeros((64, kc), np.float32)
    aqt[0:12] = aq12
    aqt[32:44] = aq12
    akt[0:12] = ak12
    akt[32:44] = ak12
    return aqt.astype(_BF16), akt.astype(_BF16)


def _stage_inputs(inputs, kc=KC):
    """Build per-core input maps (host-side sharding + key compaction)."""
    query = np.asarray(inputs["query"], np.float32)
    key_value = np.asarray(inputs["key_value"], np.float32)
    query_pos = np.asarray(inputs["query_pos"], np.float32)
    key_pos = np.asarray(inputs["key_pos"], np.float32)
    key_mask = np.asarray(inputs["key_mask"]).astype(bool)

    wqT = np.ascontiguousarray((np.asarray(inputs["Wq"], np.float32) * SCALE).T
                               ).astype(_BF16)
    wkT = np.ascontiguousarray(np.asarray(inputs["Wk"], np.float32).T).astype(_BF16)
    wvT = np.ascontiguousarray(np.asarray(inputs["Wv"], np.float32).T).astype(_BF16)
    woT = np.ascontiguousarray(np.asarray(inputs["Wo"], np.float32).T).astype(_BF16)
    bq2 = np.ascontiguousarray(
        (np.asarray(inputs["bq"], np.float32) * SCALE).reshape(4, 128).T)
    bk2 = np.ascontiguousarray(np.asarray(inputs["bk"], np.float32).reshape(4, 128).T)
    # fold V bias through the out-projection (attn rows sum to 1):
    # out = attn@(v+bv) @ Wo^T + bo = attn@v @ Wo^T + (bv @ Wo^T + bo)
    bo_eff = (np.asarray(inputs["bv"], np.float32)
              @ np.asarray(inputs["Wo"], np.float32).T
              + np.asarray(inputs["bo"], np.float32))
    bob = np.ascontiguousarray(np.broadcast_to(bo_eff, (128, D)))

    in_maps = []
    for b in range(B):
        idx = np.nonzero(key_mask[b])[0]
        nv = len(idx)
        assert nv <= kc, f"batch {b}: {nv} valid keys > kc={kc}"
        kvc = np.zeros((N, kc, D), np.float32)
        kvc[:, :nv, :] = key_value[b][:, idx, :]
        kvT = np.ascontiguousarray(kvc.reshape(N * kc, D).T).astype(_BF16)
        qT = np.ascontiguousarray(query[b].reshape(N * Q, D).T).astype(_BF16)
        aq12, ak12 = _bias_factors(query_pos[b], key_pos[b][idx], nv, kc)
        in_maps.append({
            "kvT": kvT, "qT": qT,
            "wqT": wqT, "wkT": wkT, "wvT": wvT, "woT": woT,
            "bq2": bq2, "bk2": bk2, "bob": bob,
            "aq12": np.ascontiguousarray(aq12),
            "ak12": np.ascontiguousarray(ak12),
        })
    return in_maps


def _pick_kc(inputs):
    km = np.asarray(inputs["key_mask"]).astype(bool)
    need = int(km.sum(1).max())
    kcv = max(KC, ((need + 127) // 128) * 128)
    return kcv


def _get_runner(kc=KC):
    """Compile (once) and return a callable in_maps -> list of out arrays."""
    ck = ("runner", kc)
    if ck in _CACHE:
        return _CACHE[ck]

    import jax
    import jax.numpy as jnp
    from jax.sharding import Mesh, PartitionSpec
    from jax.experimental.shard_map import shard_map
    from concourse import bass2jax
    from concourse.bass2jax import (_bass_exec_p, install_neuronx_cc_hook,
                                    partition_id_tensor)
    import concourse.mybir as mybir

    nc = _build_program(kc=kc)
    install_neuronx_cc_hook()

    in_names = ["kvT", "qT", "wqT", "wkT", "wvT", "woT",
                "bq2", "bk2", "bob", "aq12", "ak12"]
    out_shape = (N, Q, D)
    out_aval = jax.core.ShapedArray(out_shape, np.float32)
    all_names = in_names + ["out", "partition_id"]

    def _body(*args):
        outs = _bass_exec_p.bind(
            *args, partition_id_tensor(),
            out_avals=(out_aval,),
            in_names=tuple(all_names),
            out_names=("out",),
            lowering_input_output_aliases=(),
            sim_require_finite=True,
            sim_require_nnan=True,
            nc=nc,
        )
        return tuple(outs)

    n_in = len(in_names)
    devices = jax.devices()[:NCORES]
    mesh = Mesh(np.asarray(devices), ("core",))
    sharded = jax.jit(
        shard_map(_body, mesh=mesh,
                  in_specs=(PartitionSpec("core"),) * (n_in + 1),
                  out_specs=(PartitionSpec("core"),),
                  check_rep=False),
        donate_argnums=(n_in,), keep_unused=True)

    def runner(in_maps):
        concat_in = [np.concatenate([np.asarray(m[name]) for m in in_maps], axis=0)
                     for name in in_names]
        zeros = np.zeros((NCORES * N, Q, D), np.float32)
        (out,) = sharded(*concat_in, zeros)
        out = np.asarray(out).reshape(NCORES, N, Q, D)
        return out

    _CACHE[ck] = runner
    _CACHE["sharded"] = sharded
    _CACHE["mesh"] = mesh
    _CACHE["in_names"] = in_names
    _CACHE["nc"] = nc
    return runner


def kernel(**inputs):
    kc = _pick_kc(inputs)
    runner = _get_runner(kc)
    in_maps = _stage_inputs(inputs, kc)
    out = runner(in_maps)          # [8 cores = B, N, Q, D]
    return np.ascontiguousarray(out)


# revision 14
# speedup vs baseline: 1.1157x; 1.1157x over previous
"""Trainium2 Bass kernel for MultiHeadCrossAttention (B=8,N=8,Q=128,K=1024,D=512,H=8).

Sharding: data-parallel over batch B — core i handles batch i.

Optimizations vs the naive form:
  1. Key compaction: `key_mask` is known at staging time and zeroes ~50% of
     keys, so the host compacts K=1024 -> KC=576 (max valid count is 550 for
     these inputs; KC re-picked at runtime if ever larger).  K-proj, V-proj,
     QK and AV all shrink by KC/K with no precision loss.
  2. The Gaussian distance bias is injected into the logits PSUM by a rank-12
     bf16 matmul: -d^2/(2s^2) = pq.pk/s^2 - |pk|^2/(2s^2) - |pq|^2/(2s^2);
     the per-query term is dropped (softmax is shift-invariant) and the
     remaining rank-4 bilinear form is expanded hi/lo (3 cross products of
     bf16 splits) for f32-faithful accuracy.  Padded key slots get a -1000
     bias column -> exp gives exactly 0.  This removes the big [Q,KC]
     exp-bias multiply on VectorE and lets the exp run straight out of PSUM
     with a fused row-sum (accum_out), so softmax sums are free.
  3. V bias is folded into the output-projection bias on the host
     (attn rows sum to 1), so the V-proj PSUM evacuation is a plain copy.
  4. Attention rows are transposed for AV with one DMA-xbar transpose per
     head (padded to 640 cols so key chunks stay 128-aligned), alternating
     between the two HWDGE queues (sync/scalar) so the transposes ride
     otherwise-idle DMA paths.

Per-core dataflow (matmuls bf16 on TensorE, f32 PSUM accumulate):
  - host stages compacted transposed bf16 activations (kvT [D, N*KC],
    qT [D, N*Q]) and transposed bf16 weights; SCALE folded into Wq/bq;
    rank-12 bias factors aq12 [12, Q] / ak12 [12, KC] duplicated at
    partition 32 for row-group-concurrent matmuls.
  - Q-proj once up front -> qTp [j, m] (transposed layout, heads on partitions)
  - per step n: K-proj -> kT [j, k] (transposed), V-proj -> v [k, j] (natural)
  - per head pair: logits = qT'^T @ kT + rank-12 bias (PSUM, natural [q, k]);
    exp on ScalarE with fused row-sum; normalize by 1/rowsum on VectorE;
    DMA-xbar transpose attn -> [k, q]; AV accumulated over key chunks
    (col-group interleaved pairs), delayed one step to hide the transpose.
  - out-proj: 4 accumulating matmuls, bias added during final evacuation.
"""

import numpy as np
import ml_dtypes

B, N, Q, K, D, H = 8, 8, 128, 1024, 512, 8
HD = D // H
SCALE = HD ** -0.5
SIGMA2 = max(0.35 * 0.35, 1e-6)
NCORES = 8
KC = 640           # compacted key count (multiple of 128; >= max valid keys)

_BF16 = ml_dtypes.bfloat16

_CACHE = {}


def _build_program(repeat=1, kc=KC):
    import concourse.bass as bass
    import concourse.mybir as mybir
    import concourse.tile as tile
    from concourse import bacc

    f32 = mybir.dt.float32
    bf16 = mybir.dt.bfloat16
    AF = mybir.ActivationFunctionType
    ALU = mybir.AluOpType

    # key-axis chunking
    kb_chunks = [(0, 512), (512, kc - 512)] if kc > 512 else [(0, kc)]
    nvc = (kc + 127) // 128          # 128-key chunks (last may be partial)
    last_w = kc - 128 * (nvc - 1)    # valid width of last chunk
    ph = 128 * nvc                   # per-head padded attn width (transpose)

    nc = bacc.Bacc("TRN2", target_bir_lowering=False, debug=False,
                   num_devices=NCORES)

    kvT_h = nc.declare_dram_parameter("kvT", [D, N * kc], bf16, isOutput=False)
    qT_h = nc.declare_dram_parameter("qT", [D, N * Q], bf16, isOutput=False)
    wq_h = nc.declare_dram_parameter("wqT", [D, D], bf16, isOutput=False)
    wk_h = nc.declare_dram_parameter("wkT", [D, D], bf16, isOutput=False)
    wv_h = nc.declare_dram_parameter("wvT", [D, D], bf16, isOutput=False)
    wo_h = nc.declare_dram_parameter("woT", [D, D], bf16, isOutput=False)
    bq_h = nc.declare_dram_parameter("bq2", [128, 4], f32, isOutput=False)
    bk_h = nc.declare_dram_parameter("bk2", [128, 4], f32, isOutput=False)
    bo_h = nc.declare_dram_parameter("bob", [128, D], f32, isOutput=False)
    aq_h = nc.declare_dram_parameter("aq12", [64, Q], bf16, isOutput=False)
    ak_h = nc.declare_dram_parameter("ak12", [64, kc], bf16, isOutput=False)
    out_h = nc.declare_dram_parameter("out", [N, Q, D], f32, isOutput=True)

    kvT = kvT_h.ap().rearrange("(c p) m -> p c m", p=128)   # [128, 4, N*kc]
    qT = qT_h.ap().rearrange("(c p) m -> p c m", p=128)     # [128, 4, N*Q]
    w_aps = {k: h.ap().rearrange("(c p) j -> p c j", p=128)
             for k, h in (("wq", wq_h), ("wk", wk_h), ("wv", wv_h), ("wo", wo_h))}
    out_ap = out_h.ap()

    with tile.TileContext(nc) as tc:
        with (
            tc.tile_pool(name="const", bufs=1) as cpool,
            tc.tile_pool(name="kvin", bufs=3) as kvpool,
            tc.tile_pool(name="kt", bufs=2) as ktpool,
            tc.tile_pool(name="vt", bufs=3) as vtpool,
            tc.tile_pool(name="attn", bufs=4) as apool,
            tc.tile_pool(name="abT", bufs=9) as tpool,
            tc.tile_pool(name="small", bufs=18) as spool,
            tc.tile_pool(name="oav", bufs=2) as opool,
            tc.tile_pool(name="pp", bufs=2, space="PSUM") as pp,
            tc.tile_pool(name="pl", bufs=2, space="PSUM") as pl,
            tc.tile_pool(name="pav", bufs=2, space="PSUM") as pav,
        ):
            # ---- constants (critical-path loads first) ----
            w = {}
            for name in ("wq", "wk", "wv", "wo"):
                w[name] = cpool.tile([128, 4, D], bf16, tag=name, name=name)
            qin = cpool.tile([128, 4, N * Q], bf16, tag="qin", name="qin")
            nc.gpsimd.dma_start(out=w["wq"][:], in_=w_aps["wq"][:])
            for mb in range(2):
                nc.gpsimd.dma_start(out=qin[:, :, mb * 512:(mb + 1) * 512],
                                    in_=qT[:, :, mb * 512:(mb + 1) * 512])
            bq2 = cpool.tile([128, 4], f32, tag="bq2", name="bq2")
            nc.gpsimd.dma_start(out=bq2[:], in_=bq_h.ap()[:])
            nc.gpsimd.dma_start(out=w["wk"][:], in_=w_aps["wk"][:])
            bk2 = cpool.tile([128, 4], f32, tag="bk2", name="bk2")
            nc.gpsimd.dma_start(out=bk2[:], in_=bk_h.ap()[:])
            nc.gpsimd.dma_start(out=w["wv"][:], in_=w_aps["wv"][:])
            aq12 = cpool.tile([64, Q], bf16, tag="aq12", name="aq12")
            nc.gpsimd.dma_start(out=aq12[:], in_=aq_h.ap()[:])
            ak12 = cpool.tile([64, kc], bf16, tag="ak12", name="ak12")
            nc.gpsimd.dma_start(out=ak12[:], in_=ak_h.ap()[:])
            bob = cpool.tile([128, D], f32, tag="bob", name="bob")

            # ---- Q projection for all steps ----
            qTp = cpool.tile([128, 4, N * Q], bf16, tag="qTp", name="qTp")

            def qproj_unit(jc, mb):
                ps = pp.tile([128, 512], f32, tag="pp", name="pp")
                for ic in range(4):
                    nc.tensor.matmul(
                        ps[:],
                        w["wq"][:, ic, jc * 128:(jc + 1) * 128],
                        qin[:, ic, mb * 512:(mb + 1) * 512],
                        start=(ic == 0), stop=(ic == 3),
                    )
                nc.scalar.activation(
                    out=qTp[:, jc, mb * 512:(mb + 1) * 512], in_=ps[:],
                    func=AF.Identity, bias=bq2[:, jc:jc + 1])

            for jc in range(4):
                for mb in range(2):
                    qproj_unit(jc, mb)

            def load_kv(n, parts=2):
                t = kvpool.tile([128, 4, kc], bf16, tag="kvin", name=f"kvin{n}")
                step = kc // parts
                for hh in range(parts):
                    nc.gpsimd.dma_start(
                        out=t[:, :, hh * step:(hh + 1) * step],
                        in_=kvT[:, :, n * kc + hh * step:n * kc + (hh + 1) * step])
                return t

            def kproj_unit(kvin, kt, jc, mb):
                off, wdt = kb_chunks[mb]
                ps = pp.tile([128, 512], f32, tag="pp", name="pp")
                for ic in range(4):
                    nc.tensor.matmul(
                        ps[:, 0:wdt],
                        w["wk"][:, ic, jc * 128:(jc + 1) * 128],
                        kvin[:, ic, off:off + wdt],
                        start=(ic == 0), stop=(ic == 3),
                    )
                nc.scalar.activation(
                    out=kt[:, jc, off:off + wdt], in_=ps[:, 0:wdt],
                    func=AF.Identity, bias=bk2[:, jc:jc + 1])

            def vproj_unit(kvin, vt, mc):
                wdt = 128 if mc < nvc - 1 else last_w
                ps = pp.tile([128, 512], f32, tag="pp", name="pp")
                for ic in range(4):
                    nc.tensor.matmul(
                        ps[0:wdt, :],
                        kvin[:, ic, mc * 128:mc * 128 + wdt],
                        w["wv"][:, ic, :],
                        start=(ic == 0), stop=(ic == 3),
                    )
                nc.vector.tensor_copy(out=vt[0:wdt, mc, :], in_=ps[0:wdt, :])

            def kproj(kvin):
                kt = ktpool.tile([128, 4, kc], bf16, tag="kt", name="kt")
                for jc in range(4):
                    for mb in range(len(kb_chunks)):
                        kproj_unit(kvin, kt, jc, mb)
                return kt

            def vproj(kvin):
                vt = vtpool.tile([128, nvc, D], bf16, tag="vt", name="vt")
                for mc in range(nvc):
                    vproj_unit(kvin, vt, mc)
                return vt

            def qk_softmax_pair(n, c, kt):
                """QK + rank-12 bias for head pair (2c, 2c+1) with
                row-group-interleaved matmuls, exp with fused row-sum,
                1/sum normalize; one xbar transpose per head (the two
                heads alternate between the sync and scalar HWDGE rings)."""
                psls = [pl.tile([Q, 1024], f32, tag="pl", name="pl")
                        for _ in range(2)]
                for off, wdt in kb_chunks:
                    for par in range(2):
                        e = par * 64
                        nc.tensor.matmul(
                            psls[par][:, off:off + wdt],
                            qTp[e:e + 64, c, n * Q:(n + 1) * Q],
                            kt[e:e + 64, c, off:off + wdt],
                            start=True, stop=False,
                            skip_group_check=True,
                        )
                for off, wdt in kb_chunks:
                    for par in range(2):
                        b0 = par * 32
                        nc.tensor.matmul(
                            psls[par][:, off:off + wdt],
                            aq12[b0:b0 + 12, :],
                            ak12[b0:b0 + 12, off:off + wdt],
                            start=False, stop=True,
                            skip_group_check=True,
                        )
                abTs = []
                for par in range(2):
                    ab = apool.tile([Q, kc], bf16, tag="ab", name="ab")
                    sums = spool.tile([Q, 1], f32, tag="sums", name="sums")
                    nc.scalar.activation(out=ab[:], in_=psls[par][:, 0:kc],
                                         func=AF.Exp, accum_out=sums[:])
                    rec = spool.tile([Q, 1], f32, tag="rec", name="rec")
                    nc.vector.reciprocal(rec[:], sums[:])
                    abn = apool.tile([Q, ph], bf16, tag="abn", name="abn")
                    nc.vector.tensor_scalar_mul(
                        out=abn[:, 0:kc], in0=ab[:], scalar1=rec[:])
                    abT = tpool.tile([128, nvc, Q], bf16, tag="abT",
                                     name="abT")
                    nc.sync.dma_start_transpose(abT[:], abn[:])
                    abTs.append(abT)
                return abTs

            def av_pair(hc, abT2, vt, oavT):
                """AV for head pair (2hc, 2hc+1), col-group interleaved."""
                psav = pav.tile([128, Q], f32, tag="pav", name="pav")
                for cidx in range(nvc):
                    wdt = 128 if cidx < nvc - 1 else last_w
                    for par in range(2):
                        h = 2 * hc + par
                        e = par * 64
                        nc.tensor.matmul(
                            psav[e:e + HD, :],
                            vt[0:wdt, cidx, h * HD:(h + 1) * HD],
                            abT2[par][0:wdt, cidx, :],
                            start=(cidx == 0), stop=(cidx == nvc - 1),
                            skip_group_check=True,
                        )
                nc.vector.tensor_copy(
                    out=oavT[:, hc, :], in_=psav[:])

            def outproj(n, oavT):
                pso = pp.tile([Q, D], f32, tag="pp", name="pp")
                for jc in range(4):
                    nc.tensor.matmul(
                        pso[:], oavT[:, jc, :], w["wo"][:, jc, :],
                        start=(jc == 0), stop=(jc == 3))
                osb = opool.tile([Q, D], f32, tag="osb", name="osb")
                nc.vector.scalar_tensor_tensor(
                    out=osb[:], in0=pso[:], scalar=1.0, in1=bob[:],
                    op0=ALU.mult, op1=ALU.add)
                nc.gpsimd.dma_start(out=out_ap[n], in_=osb[:])

            # ---- software-pipelined steps (AV delayed one step) ----
            for _rep in range(repeat):
                kv0 = load_kv(0, parts=4)
                kv1 = load_kv(1)
                kvs = {0: kv0, 1: kv1}
                kts = {0: kproj(kv0)}
                vts = {0: vproj(kv0)}
                abTs_prev = None
                for n in range(N):
                    if n + 2 < N:
                        kvs[n + 2] = load_kv(n + 2)
                    if _rep == 0 and n == 0:
                        nc.gpsimd.dma_start(out=w["wo"][:], in_=w_aps["wo"][:])
                        nc.gpsimd.dma_start(out=bob[:], in_=bo_h.ap()[:])
                    if n + 1 < N:
                        ktn = ktpool.tile([128, 4, kc], bf16, tag="kt",
                                          name="kt")
                        vtn = vtpool.tile([128, nvc, D], bf16, tag="vt",
                                          name="vt")
                        proj_units = (
                            [(kproj_unit, (kvs[n + 1], ktn, jc, mb))
                             for jc in range(4) for mb in range(len(kb_chunks))]
                            + [(vproj_unit, (kvs[n + 1], vtn, mc))
                               for mc in range(nvc)])
                        kts[n + 1] = ktn
                        vts[n + 1] = vtn
                    else:
                        proj_units = []
                    oavT_prev = None
                    if abTs_prev is not None:
                        oavT_prev = opool.tile([128, 4, Q], bf16,
                                               tag="oavT", name="oavT")
                    abTs = []
                    pu = 0
                    for hc in range(4):
                        abTs.append(qk_softmax_pair(n, hc, kts[n]))
                        if oavT_prev is not None:
                            av_pair(hc, abTs_prev[hc], vts[n - 1], oavT_prev)
                        take = 3 if hc < 3 else len(proj_units) - pu
                        for _ in range(max(0, take)):
                            if pu < len(proj_units):
                                fn, args = proj_units[pu]
                                fn(*args)
                                pu += 1
                    if oavT_prev is not None:
                        outproj(n - 1, oavT_prev)
                    abTs_prev = abTs
                # epilogue: AV + out-proj for the last step
                oavT_last = opool.tile([128, 4, Q], bf16, tag="oavT",
                                       name="oavT")
                for hc in range(4):
                    av_pair(hc, abTs_prev[hc], vts[N - 1], oavT_last)
                outproj(N - 1, oavT_last)

    nc.compile()
    return nc


def _bias_factors(query_pos, key_pos_valid, nv, kc):
    """Rank-12 bf16 factorization of the Gaussian bias (per-query term
    dropped; softmax-invariant).  Returns aq12 [64, Q], ak12 [64, kc]."""
    pq = query_pos - 0.5                     # [Q, 3] centered
    pk = key_pos_valid - 0.5                 # [nv, 3]
    aq4 = np.zeros((4, Q), np.float32)
    aq4[0:3] = pq.T / SIGMA2
    aq4[3] = 1.0
    ak4 = np.zeros((4, kc), np.float32)
    ak4[0:3, :nv] = pk.T
    ak4[3, :nv] = -(pk ** 2).sum(-1) / (2.0 * SIGMA2)
    ak4[3, nv:] = -1000.0                    # padded keys -> exp() == 0
    aq_hi = aq4.astype(_BF16).astype(np.float32)
    aq_lo = (aq4 - aq_hi).astype(_BF16).astype(np.float32)
    ak_hi = ak4.astype(_BF16).astype(np.float32)
    ak_lo = (ak4 - ak_hi).astype(_BF16).astype(np.float32)
    # bias ~= hi.hi + hi.lo + lo.hi
    aq12 = np.concatenate([aq_hi, aq_hi, aq_lo], 0)     # [12, Q]
    ak12 = np.concatenate([ak_hi, ak_lo, ak_hi], 0)     # [12, kc]
    aqt = np.zeros((64, Q), np.float32)
    akt = np.zeros((64, kc), np.float32)
    aqt[0:12] = aq12
    aqt[32:44] = aq12
    akt[0:12] = ak12
    akt[32:44] = ak12
    return aqt.astype(_BF16), akt.astype(_BF16)


def _stage_inputs(inputs, kc=KC):
    """Build per-core input maps (host-side sharding + key compaction)."""
    query = np.asarray(inputs["query"], np.float32)
    key_value = np.asarray(inputs["key_value"], np.float32)
    query_pos = np.asarray(inputs["query_pos"], np.float32)
    key_pos = np.asarray(inputs["key_pos"], np.float32)
    key_mask = np.asarray(inputs["key_mask"]).astype(bool)

    wqT = np.ascontiguousarray((np.asarray(inputs["Wq"], np.float32) * SCALE).T
                               ).astype(_BF16)
    wkT = np.ascontiguousarray(np.asarray(inputs["Wk"], np.float32).T).astype(_BF16)
    wvT = np.ascontiguousarray(np.asarray(inputs["Wv"], np.float32).T).astype(_BF16)
    woT = np.ascontiguousarray(np.asarray(inputs["Wo"], np.float32).T).astype(_BF16)
    bq2 = np.ascontiguousarray(
        (np.asarray(inputs["bq"], np.float32) * SCALE).reshape(4, 128).T)
    bk2 = np.ascontiguousarray(np.asarray(inputs["bk"], np.float32).reshape(4, 128).T)
    # fold V bias through the out-projection (attn rows sum to 1):
    # out = attn@(v+bv) @ Wo^T + bo = attn@v @ Wo^T + (bv @ Wo^T + bo)
    bo_eff = (np.asarray(inputs["bv"], np.float32)
              @ np.asarray(inputs["Wo"], np.float32).T
              + np.asarray(inputs["bo"], np.float32))
    bob = np.ascontiguousarray(np.broadcast_to(bo_eff, (128, D)))

    in_maps = []
    for b in range(B):
        idx = np.nonzero(key_mask[b])[0]
        nv = len(idx)
        assert nv <= kc, f"batch {b}: {nv} valid keys > kc={kc}"
        kvc = np.zeros((N, kc, D), np.float32)
        kvc[:, :nv, :] = key_value[b][:, idx, :]
        kvT = np.ascontiguousarray(kvc.reshape(N * kc, D).T).astype(_BF16)
        qT = np.ascontiguousarray(query[b].reshape(N * Q, D).T).astype(_BF16)
        aq12, ak12 = _bias_factors(query_pos[b], key_pos[b][idx], nv, kc)
        in_maps.append({
            "kvT": kvT, "qT": qT,
            "wqT": wqT, "wkT": wkT, "wvT": wvT, "woT": woT,
            "bq2": bq2, "bk2": bk2, "bob": bob,
            "aq12": np.ascontiguousarray(aq12),
            "ak12": np.ascontiguousarray(ak12),
        })
    return in_maps


def _pick_kc(inputs):
    km = np.asarray(inputs["key_mask"]).astype(bool)
    need = int(km.sum(1).max())
    kcv = max(KC, ((need + 127) // 128) * 128)
    return kcv


def _get_runner(kc=KC):
    """Compile (once) and return a callable in_maps -> list of out arrays."""
    ck = ("runner", kc)
    if ck in _CACHE:
        return _CACHE[ck]

    import jax
    import jax.numpy as jnp
    from jax.sharding import Mesh, PartitionSpec
    from jax.experimental.shard_map import shard_map
    from concourse import bass2jax
    from concourse.bass2jax import (_bass_exec_p, install_neuronx_cc_hook,
                                    partition_id_tensor)
    import concourse.mybir as mybir

    nc = _build_program(kc=kc)
    install_neuronx_cc_hook()

    in_names = ["kvT", "qT", "wqT", "wkT", "wvT", "woT",
                "bq2", "bk2", "bob", "aq12", "ak12"]
    out_shape = (N, Q, D)
    out_aval = jax.core.ShapedArray(out_shape, np.float32)
    all_names = in_names + ["out", "partition_id"]

    def _body(*args):
        outs = _bass_exec_p.bind(
            *args, partition_id_tensor(),
            out_avals=(out_aval,),
            in_names=tuple(all_names),
            out_names=("out",),
            lowering_input_output_aliases=(),
            sim_require_finite=True,
            sim_require_nnan=True,
            nc=nc,
        )
        return tuple(outs)

    n_in = len(in_names)
    devices = jax.devices()[:NCORES]
    mesh = Mesh(np.asarray(devices), ("core",))
    sharded = jax.jit(
        shard_map(_body, mesh=mesh,
                  in_specs=(PartitionSpec("core"),) * (n_in + 1),
                  out_specs=(PartitionSpec("core"),),
                  check_rep=False),
        donate_argnums=(n_in,), keep_unused=True)

    def runner(in_maps):
        concat_in = [np.concatenate([np.asarray(m[name]) for m in in_maps], axis=0)
                     for name in in_names]
        zeros = np.zeros((NCORES * N, Q, D), np.float32)
        (out,) = sharded(*concat_in, zeros)
        out = np.asarray(out).reshape(NCORES, N, Q, D)
        return out

    _CACHE[ck] = runner
    _CACHE["sharded"] = sharded
    _CACHE["mesh"] = mesh
    _CACHE["in_names"] = in_names
    _CACHE["nc"] = nc
    return runner


def kernel(**inputs):
    kc = _pick_kc(inputs)
    runner = _get_runner(kc)
    in_maps = _stage_inputs(inputs, kc)
    out = runner(in_maps)          # [8 cores = B, N, Q, D]
    return np.ascontiguousarray(out)


# revision 15
# speedup vs baseline: 1.4819x; 1.3283x over previous
"""Trainium2 Bass kernel for MultiHeadCrossAttention (B=8,N=8,Q=128,K=1024,D=512,H=8).

Sharding: data-parallel over batch B — core i handles batch i.

Optimizations vs the naive form:
  1. Key compaction: `key_mask` is known at staging time and zeroes ~50% of
     keys, so the host compacts K=1024 -> KC=576 (max valid count is 550 for
     these inputs; KC re-picked at runtime if ever larger).  K-proj, V-proj,
     QK and AV all shrink by KC/K with no precision loss.
  2. The Gaussian distance bias is injected into the logits PSUM by a rank-12
     bf16 matmul: -d^2/(2s^2) = pq.pk/s^2 - |pk|^2/(2s^2) - |pq|^2/(2s^2);
     the per-query term is dropped (softmax is shift-invariant) and the
     remaining rank-4 bilinear form is expanded hi/lo (3 cross products of
     bf16 splits) for f32-faithful accuracy.  Padded key slots get a -1000
     bias column -> exp gives exactly 0.  This removes the big [Q,KC]
     exp-bias multiply on VectorE and lets the exp run straight out of PSUM
     with a fused row-sum (accum_out), so softmax sums are free.
  3. V bias is folded into the output-projection bias on the host
     (attn rows sum to 1), so the V-proj PSUM evacuation is a plain copy.
  4. Attention rows are transposed for AV with one DMA-xbar transpose per
     head (padded to 640 cols so key chunks stay 128-aligned), alternating
     between the two HWDGE queues (sync/scalar) so the transposes ride
     otherwise-idle DMA paths.

Per-core dataflow (matmuls bf16 on TensorE, f32 PSUM accumulate):
  - host stages compacted transposed bf16 activations (kvT [D, N*KC],
    qT [D, N*Q]) and transposed bf16 weights; SCALE folded into Wq/bq;
    rank-12 bias factors aq12 [12, Q] / ak12 [12, KC] duplicated at
    partition 32 for row-group-concurrent matmuls.
  - Q-proj once up front -> qTp [j, m] (transposed layout, heads on partitions)
  - per step n: K-proj -> kT [j, k] (transposed), V-proj -> v [k, j] (natural)
  - per head pair: logits = qT'^T @ kT + rank-12 bias (PSUM, natural [q, k]);
    exp on ScalarE with fused row-sum; normalize by 1/rowsum on VectorE;
    DMA-xbar transpose attn -> [k, q]; AV accumulated over key chunks
    (col-group interleaved pairs), delayed one step to hide the transpose.
  - out-proj: 4 accumulating matmuls, bias added during final evacuation.
"""

import numpy as np
import ml_dtypes

B, N, Q, K, D, H = 8, 8, 128, 1024, 512, 8
HD = D // H
SCALE = HD ** -0.5
SIGMA2 = max(0.35 * 0.35, 1e-6)
NCORES = 8
KC = 640           # compacted key count (multiple of 128; >= max valid keys)

_BF16 = ml_dtypes.bfloat16

_CACHE = {}


def _build_program(repeat=1, kc=KC):
    import concourse.bass as bass
    import concourse.mybir as mybir
    import concourse.tile as tile
    from concourse import bacc

    f32 = mybir.dt.float32
    bf16 = mybir.dt.bfloat16
    AF = mybir.ActivationFunctionType
    ALU = mybir.AluOpType

    # key-axis chunking
    kb_chunks = [(0, 512), (512, kc - 512)] if kc > 512 else [(0, kc)]
    nvc = (kc + 127) // 128          # 128-key chunks (last may be partial)
    last_w = kc - 128 * (nvc - 1)    # valid width of last chunk
    ph = 128 * nvc                   # per-head padded attn width (transpose)

    nc = bacc.Bacc("TRN2", target_bir_lowering=False, debug=False,
                   num_devices=NCORES)

    kvT_h = nc.declare_dram_parameter("kvT", [D, N * kc], bf16, isOutput=False)
    qT_h = nc.declare_dram_parameter("qT", [D, N * Q], bf16, isOutput=False)
    wq_h = nc.declare_dram_parameter("wqT", [D, D], bf16, isOutput=False)
    wk_h = nc.declare_dram_parameter("wkT", [D, D], bf16, isOutput=False)
    wv_h = nc.declare_dram_parameter("wvT", [D, D], bf16, isOutput=False)
    wo_h = nc.declare_dram_parameter("woT", [D, D], bf16, isOutput=False)
    bq_h = nc.declare_dram_parameter("bq2", [128, 4], f32, isOutput=False)
    bk_h = nc.declare_dram_parameter("bk2", [128, 4], f32, isOutput=False)
    bo_h = nc.declare_dram_parameter("bob", [128, D], f32, isOutput=False)
    aq_h = nc.declare_dram_parameter("aq12", [64, Q], bf16, isOutput=False)
    ak_h = nc.declare_dram_parameter("ak12", [64, kc], bf16, isOutput=False)
    out_h = nc.declare_dram_parameter("out", [N, Q, D], f32, isOutput=True)

    kvT = kvT_h.ap().rearrange("(c p) m -> p c m", p=128)   # [128, 4, N*kc]
    qT = qT_h.ap().rearrange("(c p) m -> p c m", p=128)     # [128, 4, N*Q]
    w_aps = {k: h.ap().rearrange("(c p) j -> p c j", p=128)
             for k, h in (("wq", wq_h), ("wk", wk_h), ("wv", wv_h), ("wo", wo_h))}
    out_ap = out_h.ap()

    with tile.TileContext(nc) as tc:
        with (
            tc.tile_pool(name="const", bufs=1) as cpool,
            tc.tile_pool(name="kvin", bufs=3) as kvpool,
            tc.tile_pool(name="kt", bufs=2) as ktpool,
            tc.tile_pool(name="vt", bufs=3) as vtpool,
            tc.tile_pool(name="attn", bufs=6) as apool,
            tc.tile_pool(name="abT", bufs=17) as tpool,
            tc.tile_pool(name="small", bufs=18) as spool,
            tc.tile_pool(name="oav", bufs=2) as opool,
            tc.tile_pool(name="pp", bufs=2, space="PSUM") as pp,
            tc.tile_pool(name="pl", bufs=2, space="PSUM") as pl,
            tc.tile_pool(name="pav", bufs=2, space="PSUM") as pav,
        ):
            # ---- constants (critical-path loads first) ----
            w = {}
            for name in ("wq", "wk", "wv", "wo"):
                w[name] = cpool.tile([128, 4, D], bf16, tag=name, name=name)
            qin = cpool.tile([128, 4, N * Q], bf16, tag="qin", name="qin")
            nc.gpsimd.dma_start(out=w["wq"][:], in_=w_aps["wq"][:])
            for mb in range(2):
                nc.gpsimd.dma_start(out=qin[:, :, mb * 512:(mb + 1) * 512],
                                    in_=qT[:, :, mb * 512:(mb + 1) * 512])
            bq2 = cpool.tile([128, 4], f32, tag="bq2", name="bq2")
            nc.gpsimd.dma_start(out=bq2[:], in_=bq_h.ap()[:])
            nc.gpsimd.dma_start(out=w["wk"][:], in_=w_aps["wk"][:])
            bk2 = cpool.tile([128, 4], f32, tag="bk2", name="bk2")
            nc.gpsimd.dma_start(out=bk2[:], in_=bk_h.ap()[:])
            nc.gpsimd.dma_start(out=w["wv"][:], in_=w_aps["wv"][:])
            aq12 = cpool.tile([64, Q], bf16, tag="aq12", name="aq12")
            nc.gpsimd.dma_start(out=aq12[:], in_=aq_h.ap()[:])
            ak12 = cpool.tile([64, kc], bf16, tag="ak12", name="ak12")
            nc.gpsimd.dma_start(out=ak12[:], in_=ak_h.ap()[:])
            bob = cpool.tile([128, D], f32, tag="bob", name="bob")

            # ---- Q projection for all steps ----
            qTp = cpool.tile([128, 4, N * Q], bf16, tag="qTp", name="qTp")

            def qproj_unit(jc, mb):
                ps = pp.tile([128, 512], f32, tag="pp", name="pp")
                for ic in range(4):
                    nc.tensor.matmul(
                        ps[:],
                        w["wq"][:, ic, jc * 128:(jc + 1) * 128],
                        qin[:, ic, mb * 512:(mb + 1) * 512],
                        start=(ic == 0), stop=(ic == 3),
                    )
                nc.scalar.activation(
                    out=qTp[:, jc, mb * 512:(mb + 1) * 512], in_=ps[:],
                    func=AF.Identity, bias=bq2[:, jc:jc + 1])

            for jc in range(4):
                for mb in range(2):
                    qproj_unit(jc, mb)

            def load_kv(n, parts=2):
                t = kvpool.tile([128, 4, kc], bf16, tag="kvin", name=f"kvin{n}")
                step = kc // parts
                for hh in range(parts):
                    nc.gpsimd.dma_start(
                        out=t[:, :, hh * step:(hh + 1) * step],
                        in_=kvT[:, :, n * kc + hh * step:n * kc + (hh + 1) * step])
                return t

            def kproj_unit(kvin, kt, jc, mb):
                off, wdt = kb_chunks[mb]
                ps = pp.tile([128, 512], f32, tag="pp", name="pp")
                for ic in range(4):
                    nc.tensor.matmul(
                        ps[:, 0:wdt],
                        w["wk"][:, ic, jc * 128:(jc + 1) * 128],
                        kvin[:, ic, off:off + wdt],
                        start=(ic == 0), stop=(ic == 3),
                    )
                nc.scalar.activation(
                    out=kt[:, jc, off:off + wdt], in_=ps[:, 0:wdt],
                    func=AF.Identity, bias=bk2[:, jc:jc + 1])

            def vproj_unit(kvin, vt, mc):
                wdt = 128 if mc < nvc - 1 else last_w
                ps = pp.tile([128, 512], f32, tag="pp", name="pp")
                for ic in range(4):
                    nc.tensor.matmul(
                        ps[0:wdt, :],
                        kvin[:, ic, mc * 128:mc * 128 + wdt],
                        w["wv"][:, ic, :],
                        start=(ic == 0), stop=(ic == 3),
                    )
                nc.vector.tensor_copy(out=vt[0:wdt, mc, :], in_=ps[0:wdt, :])

            def kproj(kvin):
                kt = ktpool.tile([128, 4, kc], bf16, tag="kt", name="kt")
                for jc in range(4):
                    for mb in range(len(kb_chunks)):
                        kproj_unit(kvin, kt, jc, mb)
                return kt

            def vproj(kvin):
                vt = vtpool.tile([128, nvc, D], bf16, tag="vt", name="vt")
                for mc in range(nvc):
                    vproj_unit(kvin, vt, mc)
                return vt

            def qk_softmax_pair(n, c, kt):
                """QK + rank-12 bias for head pair (2c, 2c+1) with
                row-group-interleaved matmuls, exp with fused row-sum,
                1/sum normalize; one xbar transpose per head (the two
                heads alternate between the sync and scalar HWDGE rings)."""
                psls = [pl.tile([Q, 1024], f32, tag="pl", name="pl")
                        for _ in range(2)]
                for off, wdt in kb_chunks:
                    for par in range(2):
                        e = par * 64
                        nc.tensor.matmul(
                            psls[par][:, off:off + wdt],
                            qTp[e:e + 64, c, n * Q:(n + 1) * Q],
                            kt[e:e + 64, c, off:off + wdt],
                            start=True, stop=False,
                            skip_group_check=True,
                        )
                for off, wdt in kb_chunks:
                    for par in range(2):
                        b0 = par * 32
                        nc.tensor.matmul(
                            psls[par][:, off:off + wdt],
                            aq12[b0:b0 + 12, :],
                            ak12[b0:b0 + 12, off:off + wdt],
                            start=False, stop=True,
                            skip_group_check=True,
                        )
                abTs = []
                for par in range(2):
                    ab = apool.tile([Q, kc], bf16, tag="ab", name="ab")
                    sums = spool.tile([Q, 1], f32, tag="sums", name="sums")
                    nc.scalar.activation(out=ab[:], in_=psls[par][:, 0:kc],
                                         func=AF.Exp, accum_out=sums[:])
                    rec = spool.tile([Q, 1], f32, tag="rec", name="rec")
                    nc.vector.reciprocal(rec[:], sums[:])
                    abn = apool.tile([Q, ph], bf16, tag="abn", name="abn")
                    nc.vector.tensor_scalar_mul(
                        out=abn[:, 0:kc], in0=ab[:], scalar1=rec[:])
                    abT = tpool.tile([128, nvc, Q], bf16, tag="abT",
                                     name="abT")
                    nc.sync.dma_start_transpose(abT[:], abn[:])
                    abTs.append(abT)
                return abTs

            def av_pair(hc, abT2, vt, oavT):
                """AV for head pair (2hc, 2hc+1), col-group interleaved."""
                psav = pav.tile([128, Q], f32, tag="pav", name="pav")
                for cidx in range(nvc):
                    wdt = 128 if cidx < nvc - 1 else last_w
                    for par in range(2):
                        h = 2 * hc + par
                        e = par * 64
                        nc.tensor.matmul(
                            psav[e:e + HD, :],
                            vt[0:wdt, cidx, h * HD:(h + 1) * HD],
                            abT2[par][0:wdt, cidx, :],
                            start=(cidx == 0), stop=(cidx == nvc - 1),
                            skip_group_check=True,
                        )
                nc.vector.tensor_copy(
                    out=oavT[:, hc, :], in_=psav[:])

            def outproj(n, oavT):
                pso = pp.tile([Q, D], f32, tag="pp", name="pp")
                for jc in range(4):
                    nc.tensor.matmul(
                        pso[:], oavT[:, jc, :], w["wo"][:, jc, :],
                        start=(jc == 0), stop=(jc == 3))
                osb = opool.tile([Q, D], f32, tag="osb", name="osb")
                nc.vector.scalar_tensor_tensor(
                    out=osb[:], in0=pso[:], scalar=1.0, in1=bob[:],
                    op0=ALU.mult, op1=ALU.add)
                nc.gpsimd.dma_start(out=out_ap[n], in_=osb[:])

            # ---- software-pipelined steps (AV delayed one step) ----
            for _rep in range(repeat):
                kv0 = load_kv(0, parts=4)
                kv1 = load_kv(1)
                kvs = {0: kv0, 1: kv1}
                kts = {0: kproj(kv0)}
                vts = {0: vproj(kv0)}
                abTs_prev = None
                for n in range(N):
                    if n + 2 < N:
                        kvs[n + 2] = load_kv(n + 2)
                    if _rep == 0 and n == 0:
                        nc.gpsimd.dma_start(out=w["wo"][:], in_=w_aps["wo"][:])
                        nc.gpsimd.dma_start(out=bob[:], in_=bo_h.ap()[:])
                    if n + 1 < N:
                        ktn = ktpool.tile([128, 4, kc], bf16, tag="kt",
                                          name="kt")
                        vtn = vtpool.tile([128, nvc, D], bf16, tag="vt",
                                          name="vt")
                        proj_units = (
                            [(kproj_unit, (kvs[n + 1], ktn, jc, mb))
                             for jc in range(4) for mb in range(len(kb_chunks))]
                            + [(vproj_unit, (kvs[n + 1], vtn, mc))
                               for mc in range(nvc)])
                        kts[n + 1] = ktn
                        vts[n + 1] = vtn
                    else:
                        proj_units = []
                    oavT_prev = None
                    if abTs_prev is not None:
                        oavT_prev = opool.tile([128, 4, Q], bf16,
                                               tag="oavT", name="oavT")
                    abTs = []
                    pu = 0
                    for hc in range(4):
                        abTs.append(qk_softmax_pair(n, hc, kts[n]))
                        if oavT_prev is not None:
                            av_pair(hc, abTs_prev[hc], vts[n - 1], oavT_prev)
                        take = 3 if hc < 3 else len(proj_units) - pu
                        for _ in range(max(0, take)):
                            if pu < len(proj_units):
                                fn, args = proj_units[pu]
                                fn(*args)
                                pu += 1
                    if oavT_prev is not None:
                        outproj(n - 1, oavT_prev)
                    abTs_prev = abTs
                # epilogue: AV + out-proj for the last step
                oavT_last = opool.tile([128, 4, Q], bf16, tag="oavT",
                                       name="oavT")
                for hc in range(4):
                    av_pair(hc, abTs_prev[hc], vts[N - 1], oavT_last)
                outproj(N - 1, oavT_last)

    nc.compile()
    return nc


def _bias_factors(query_pos, key_pos_valid, nv, kc):
    """Rank-12 bf16 factorization of the Gaussian bias (per-query term
    dropped; softmax-invariant).  Returns aq12 [64, Q], ak12 [64, kc]."""
    pq = query_pos - 0.5                     # [Q, 3] centered
    pk = key_pos_valid - 0.5                 # [nv, 3]
    aq4 = np.zeros((4, Q), np.float32)
    aq4[0:3] = pq.T / SIGMA2
    aq4[3] = 1.0
    ak4 = np.zeros((4, kc), np.float32)
    ak4[0:3, :nv] = pk.T
    ak4[3, :nv] = -(pk ** 2).sum(-1) / (2.0 * SIGMA2)
    ak4[3, nv:] = -1000.0                    # padded keys -> exp() == 0
    aq_hi = aq4.astype(_BF16).astype(np.float32)
    aq_lo = (aq4 - aq_hi).astype(_BF16).astype(np.float32)
    ak_hi = ak4.astype(_BF16).astype(np.float32)
    ak_lo = (ak4 - ak_hi).astype(_BF16).astype(np.float32)
    # bias ~= hi.hi + hi.lo + lo.hi
    aq12 = np.concatenate([aq_hi, aq_hi, aq_lo], 0)     # [12, Q]
    ak12 = np.concatenate([ak_hi, ak_lo, ak_hi], 0)     # [12, kc]
    aqt = np.zeros((64, Q), np.float32)
    akt = np.zeros((64, kc), np.float32)
    aqt[0:12] = aq12
    aqt[32:44] = aq12
    akt[0:12] = ak12
    akt[32:44] = ak12
    return aqt.astype(_BF16), akt.astype(_BF16)


def _stage_inputs(inputs, kc=KC):
    """Build per-core input maps (host-side sharding + key compaction)."""
    query = np.asarray(inputs["query"], np.float32)
    key_value = np.asarray(inputs["key_value"], np.float32)
    query_pos = np.asarray(inputs["query_pos"], np.float32)
    key_pos = np.asarray(inputs["key_pos"], np.float32)
    key_mask = np.asarray(inputs["key_mask"]).astype(bool)

    wqT = np.ascontiguousarray((np.asarray(inputs["Wq"], np.float32) * SCALE).T
                               ).astype(_BF16)
    wkT = np.ascontiguousarray(np.asarray(inputs["Wk"], np.float32).T).astype(_BF16)
    wvT = np.ascontiguousarray(np.asarray(inputs["Wv"], np.float32).T).astype(_BF16)
    woT = np.ascontiguousarray(np.asarray(inputs["Wo"], np.float32).T).astype(_BF16)
    bq2 = np.ascontiguousarray(
        (np.asarray(inputs["bq"], np.float32) * SCALE).reshape(4, 128).T)
    bk2 = np.ascontiguousarray(np.asarray(inputs["bk"], np.float32).reshape(4, 128).T)
    # fold V bias through the out-projection (attn rows sum to 1):
    # out = attn@(v+bv) @ Wo^T + bo = attn@v @ Wo^T + (bv @ Wo^T + bo)
    bo_eff = (np.asarray(inputs["bv"], np.float32)
              @ np.asarray(inputs["Wo"], np.float32).T
              + np.asarray(inputs["bo"], np.float32))
    bob = np.ascontiguousarray(np.broadcast_to(bo_eff, (128, D)))

    in_maps = []
    for b in range(B):
        idx = np.nonzero(key_mask[b])[0]
        nv = len(idx)
        assert nv <= kc, f"batch {b}: {nv} valid keys > kc={kc}"
        kvc = np.zeros((N, kc, D), np.float32)
        kvc[:, :nv, :] = key_value[b][:, idx, :]
        kvT = np.ascontiguousarray(kvc.reshape(N * kc, D).T).astype(_BF16)
        qT = np.ascontiguousarray(query[b].reshape(N * Q, D).T).astype(_BF16)
        aq12, ak12 = _bias_factors(query_pos[b], key_pos[b][idx], nv, kc)
        in_maps.append({
            "kvT": kvT, "qT": qT,
            "wqT": wqT, "wkT": wkT, "wvT": wvT, "woT": woT,
            "bq2": bq2, "bk2": bk2, "bob": bob,
            "aq12": np.ascontiguousarray(aq12),
            "ak12": np.ascontiguousarray(ak12),
        })
    return in_maps


def _pick_kc(inputs):
    km = np.asarray(inputs["key_mask"]).astype(bool)
    need = int(km.sum(1).max())
    kcv = max(KC, ((need + 127) // 128) * 128)
    return kcv


def _get_runner(kc=KC):
    """Compile (once) and return a callable in_maps -> list of out arrays."""
    ck = ("runner", kc)
    if ck in _CACHE:
        return _CACHE[ck]

    import jax
    import jax.numpy as jnp
    from jax.sharding import Mesh, PartitionSpec
    from jax.experimental.shard_map import shard_map
    from concourse import bass2jax
    from concourse.bass2jax import (_bass_exec_p, install_neuronx_cc_hook,
                                    partition_id_tensor)
    import concourse.mybir as mybir

    nc = _build_program(kc=kc)
    install_neuronx_cc_hook()

    in_names = ["kvT", "qT", "wqT", "wkT", "wvT", "woT",
                "bq2", "bk2", "bob", "aq12", "ak12"]
    out_shape = (N, Q, D)
    out_aval = jax.core.ShapedArray(out_shape, np.float32)
    all_names = in_names + ["out", "partition_id"]

    def _body(*args):
        outs = _bass_exec_p.bind(
            *args, partition_id_tensor(),
            out_avals=(out_aval,),
            in_names=tuple(all_names),
            out_names=("out",),
            lowering_input_output_aliases=(),
            sim_require_finite=True,
            sim_require_nnan=True,
            nc=nc,
        )
        return tuple(outs)

    n_in = len(in_names)
    devices = jax.devices()[:NCORES]
    mesh = Mesh(np.asarray(devices), ("core",))
    sharded = jax.jit(
        shard_map(_body, mesh=mesh,
                  in_specs=(PartitionSpec("core"),) * (n_in + 1),
                  out_specs=(PartitionSpec("core"),),
                  check_rep=False),
        donate_argnums=(n_in,), keep_unused=True)

    def runner(in_maps):
        concat_in = [np.concatenate([np.asarray(m[name]) for m in in_maps], axis=0)
                     for name in in_names]
        zeros = np.zeros((NCORES * N, Q, D), np.float32)
        (out,) = sharded(*concat_in, zeros)
        out = np.asarray(out).reshape(NCORES, N, Q, D)
        return out

    _CACHE[ck] = runner
    _CACHE["sharded"] = sharded
    _CACHE["mesh"] = mesh
    _CACHE["in_names"] = in_names
    _CACHE["nc"] = nc
    return runner


def kernel(**inputs):
    kc = _pick_kc(inputs)
    runner = _get_runner(kc)
    in_maps = _stage_inputs(inputs, kc)
    out = runner(in_maps)          # [8 cores = B, N, Q, D]
    return np.ascontiguousarray(out)
